# revision 7
# baseline (speedup 1.0000x reference)
"""Trainium2 Bass kernel for nn_DRSolver (Douglas-Rachford QP solver).

Mathematical collapse: JF and the Hessian are batch-constant, so prox_g1 is
the affine map y = P x + C p and the DR step, in the relu-lifted state
z = [y_top(64); u(32); relu(v)(32)], is

    z' = max(Wx z + Wp p, s),   s = -inf on rows 0:96, 0 on rows 96:128

Device mapping (per core: 512 batch columns, state [128, cols] f16).
Columns split in two fixed regions so both psum-capable engines evacuate in
parallel, ONE instruction each per step:

* Region A (cols 0:AA, VectorE): SHIFTED state zh = z - m with
  m = (I-Wx)^-1 Wp p.  zh' = max(Wx zh, T), T = s - m precomputed — no
  per-step bias matmul; evacuation is one tensor_tensor(max).

* Region B (cols AA:512, ScalarE): OFFSET state zt = z + delta,
  delta = B0 on rows 0:96.  Evacuation is one activation(Relu, bias):
  bias = delta - B0*rowsum(Wx[:, :96]) makes Relu a copy on the
  (always-positive) pass rows and exact max(.,0) on the relu rows, while
  cancelling the Wx*delta term — so region B needs only Wp and Wx matmuls.

Per step: 3 matmuls + 1 DVE op + 1 ACT op.  All inputs ride ONE
gpsimd(SWDGE) DMA — the hardware-DGE queues move ~7-14 GB/s here while the
SW path aggregates packets across all 16 DMA engines (~400 GB/s).  fp32
biases are smuggled in the f16 tensor and bitcast on device.  Dummy matmuls
warm the PE clock gate and a dummy Relu preloads the ACT table during the
DMA wait.
"""

import numpy as np

import concourse.bass as bass
import concourse.tile as tile
import concourse.mybir as mybir
from concourse.bass_utils import run_bass_kernel_spmd

X_DIM, N_INEQ, N_EQ = 64, 32, 16
N = X_DIM + N_INEQ          # 96
M = N_EQ + N_INEQ           # 48
NP = X_DIM + N_EQ + N_INEQ  # 112 (parms dim)
NUM_STEPS = 10
BATCH = 4096
NCORES = 8
BPC = BATCH // NCORES       # 512 samples per core
SA = 184                    # per-chain cols for A0/A1 (VectorE, shifted state)
AA = 2 * SA                 # region A total
BB = BPC - AA               # region B (ScalarE, offset state)
B0 = 32.0                   # pass-row offset for region B (|state| < 26)
NEG_BIG = -60000.0

F32 = mybir.dt.float32
F16 = mybir.dt.float16
AF = mybir.ActivationFunctionType

# packed input tensor column layout (f16); chunk1 = everything step 1 and
# the T build need, chunk2 (W1, WfinA, biases) only matters from step 2 on.
W2B_O, W2A1_O, WFT_O, WMT_O = 0, 128, 256, 384
XT_O = 512                  # x.T           rows 0:96,  512 cols
PT_O = XT_O + BPC           # parms.T       rows 0:112, 512 cols
CHUNK1 = PT_O + BPC
W1_O, WFIN_O = CHUNK1, CHUNK1 + 128
BI_O = WFIN_O + 64          # f32 biases as f16 bit pairs: [bias2 | biasF]
IN_COLS = BI_O + 4


def _precompute(Q: np.ndarray, A: np.ndarray, G: np.ndarray) -> np.ndarray:
    """Host-side factorization collapse (float64); returns the weight+bias
    part of the packed input tensor, [128, IN_COLS] f16 with x/p zeroed."""
    Qd, Ad, Gd = (m.astype(np.float64) for m in (Q, A, G))
    JF = np.zeros((M, N))
    JF[:N_EQ, :X_DIM] = Ad
    JF[N_EQ:, :X_DIM] = Gd
    JF[N_EQ:, X_DIM:] = np.eye(N_INEQ)
    Md = np.eye(N)
    Md[:X_DIM, :X_DIM] += Qd                      # gamma/2 * I + blockdiag(Q,0)
    Qc, _ = np.linalg.qr(JF.T, mode="complete")
    Qn = Qc[:, M:]                                # null-space basis of JF
    S = Qn.T @ Md @ Qn
    P = Qn @ np.linalg.solve(S, Qn.T)
    Z = JF.T @ np.linalg.solve(JF @ JF.T, np.eye(M))  # pinv(JF)
    C = np.zeros((N, NP))
    C[:, :X_DIM] = -P[:, :X_DIM]
    C[:, X_DIM:] = Z - P @ (Md @ Z)

    Es = np.eye(N)[X_DIM:]
    Ps, Cs = P[X_DIM:], C[X_DIM:]
    Wfull = np.concatenate([P[:X_DIM], Es - Ps, 2 * Ps - Es], 0)   # [128, 96]
    Wp_c = np.concatenate([C[:X_DIM], -Cs, 2 * Cs], 0)             # [128, 112]
    L = np.zeros((96, 128))
    L[:96, :96] = np.eye(96)
    L[64:96, 96:128] = np.eye(32)
    Wx = Wfull @ L                                                 # [128, 128]
    Mp = np.linalg.solve(np.eye(128) - Wx, Wp_c)                   # [128, 112]
    wxs = Wx[:, :96].sum(1)

    w = np.zeros((128, IN_COLS), dtype=np.float64)
    w[:128, W1_O:W1_O + 128] = Wx.T
    w[:NP, W2B_O:W2B_O + 128] = Wp_c.T
    w[:NP, W2A1_O:W2A1_O + 128] = (Wp_c - Mp).T
    w[:N, WFT_O:WFT_O + 128] = Wfull.T
    w[:NP, WMT_O + 96:WMT_O + 128] = -Mp[96:128].T   # cols 0:96 stay zero
    w[:NP, WFIN_O:WFIN_O + 64] = Mp[:64].T
    wf = w.astype(np.float16)
    # fp32 biases bit-packed into f16 lanes: [bias2 | biasF]
    delta = np.concatenate([B0 * np.ones(96), np.zeros(32)])
    bias2 = (delta - B0 * wxs).astype(np.float32)       # steps 2..9 (ACT)
    biasF = (-B0 * wxs).astype(np.float32)              # final region B (DVE)
    bb = np.stack([bias2, biasF], 1)                    # [128, 2] f32
    wf[:, BI_O:BI_O + 4] = bb.view(np.float16)
    return wf


def _build_nc() -> bass.Bass:
    nc = bass.Bass()
    in_d = nc.dram_tensor("inp", [128, IN_COLS], F16, kind="ExternalInput")
    yt_d = nc.dram_tensor("yt", [X_DIM, BPC], F16, kind="ExternalOutput")

    with tile.TileContext(nc) as tc:
        with (
            tc.tile_pool(name="sbuf", bufs=1) as cpool,
            tc.tile_pool(name="stA", bufs=2) as sApool,
            tc.tile_pool(name="stB", bufs=2) as sBpool,
            tc.tile_pool(name="psA", bufs=2, space="PSUM") as pApool,
            tc.tile_pool(name="psB", bufs=2, space="PSUM") as pBpool,
            tc.tile_pool(name="psC", bufs=2, space="PSUM") as pCpool,
            tc.tile_pool(name="psm", bufs=1, space="PSUM") as mpool,
        ):
            inp = cpool.tile([128, IN_COLS], F16, tag="inp")
            # SWDGE DMAs (packets aggregate across all 16 DMA engines);
            # chunk1 unblocks the T build and step 1, chunk2 follows.
            nc.gpsimd.dma_start(inp[:, :CHUNK1], in_d[:, :CHUNK1])
            nc.gpsimd.dma_start(inp[:, CHUNK1:], in_d[:, CHUNK1:])
            xt = inp[:N, XT_O:XT_O + BPC]
            pt = inp[:NP, PT_O:PT_O + BPC]
            bias2 = inp[:, BI_O:BI_O + 4].bitcast(F32)[:, 0:1]
            biasF = inp[:X_DIM, BI_O:BI_O + 4].bitcast(F32)[:, 1:2]

            # Constants (engines are free while the DMA is in flight)
            bias1 = cpool.tile([128, 1], F32, tag="bias1")
            nc.vector.memset(bias1[:N, :], B0)
            nc.vector.memset(bias1[N:, :], 0.0)
            Tt = cpool.tile([128, AA], F32, tag="Tt")
            nc.vector.memset(Tt[:N, :], NEG_BIG)

            # Dummy activation: walrus inserts the ACT table load (~2.7us)
            # before this, overlapping the input-DMA wait.
            dum = cpool.tile([1, 1], F32, tag="dum")
            nc.scalar.activation(dum[:], bias1[0:1, :], AF.Relu)

            # HAM warm-up matmuls on a zero scratch tile.
            scr = cpool.tile([128, BPC], F16, tag="scr")
            nc.gpsimd.memset(scr[:], 0.0)
            wps = mpool.tile([128, BPC], F32, tag="warm")
            for _ in range(3):
                nc.tensor.matmul(wps[:], scr[:, :128], scr[:], start=True, stop=True)

            # Region A threshold: T rows 96:128 = -m_v (WMT has zero cols 0:96)
            nc.tensor.matmul(wps[:, :AA], inp[:NP, WMT_O:WMT_O + 128],
                             pt[:, :AA], start=True, stop=True)
            nc.scalar.activation(Tt[N:, :SA], wps[N:, :SA], AF.Copy)
            nc.vector.tensor_copy(Tt[N:, SA:], wps[N:, SA:AA])

            zA0 = zA1 = zB = None
            for k in range(NUM_STEPS - 1):
                pA0 = pApool.tile([128, SA], F32, tag="pA0")
                pA1 = pBpool.tile([128, SA], F32, tag="pA1")
                pB = pCpool.tile([128, BB], F32, tag="pB")
                if k == 0:
                    # z1 = max(Wfull x + c, s); region A gets c - m instead.
                    nc.tensor.matmul(pB[:], inp[:NP, W2B_O:W2B_O + 128],
                                     pt[:, AA:], start=True, stop=False)
                    nc.tensor.matmul(pA0[:], inp[:NP, W2A1_O:W2A1_O + 128],
                                     pt[:, :SA], start=True, stop=False)
                    nc.tensor.matmul(pA1[:], inp[:NP, W2A1_O:W2A1_O + 128],
                                     pt[:, SA:AA], start=True, stop=False)
                    nc.tensor.matmul(pB[:], inp[:N, WFT_O:WFT_O + 128],
                                     xt[:, AA:], start=False, stop=True)
                    nc.tensor.matmul(pA0[:], inp[:N, WFT_O:WFT_O + 128],
                                     xt[:, :SA], start=False, stop=True)
                    nc.tensor.matmul(pA1[:], inp[:N, WFT_O:WFT_O + 128],
                                     xt[:, SA:AA], start=False, stop=True)
                    bact = bias1
                else:
                    nc.tensor.matmul(pB[:], inp[:NP, W2B_O:W2B_O + 128],
                                     pt[:, AA:], start=True, stop=False)
                    nc.tensor.matmul(pA0[:], inp[:, W1_O:W1_O + 128],
                                     zA0[:], start=True, stop=True)
                    nc.tensor.matmul(pA1[:], inp[:, W1_O:W1_O + 128],
                                     zA1[:], start=True, stop=True)
                    nc.tensor.matmul(pB[:], inp[:, W1_O:W1_O + 128],
                                     zB[:], start=False, stop=True)
                    bact = bias2
                zA0n = sApool.tile([128, SA], F16, tag="zA0")
                zA1n = sApool.tile([128, SA], F16, tag="zA1")
                zBn = sBpool.tile([128, BB], F16, tag="zB")
                nc.vector.tensor_tensor(zA0n[:], pA0[:], Tt[:, :SA],
                                        mybir.AluOpType.max)
                nc.vector.tensor_tensor(zA1n[:], pA1[:], Tt[:, SA:],
                                        mybir.AluOpType.max)
                nc.scalar.activation(zBn[:], pB[:], AF.Relu, bias=bact)
                zA0, zA1, zB = zA0n, zA1n, zBn

            # Final step: only y[:64]; region A needs +m_top (= Mp[:64] p),
            # region B needs -B0*wxs[:64] (DVE tensor_scalar, per-partition).
            fA0 = pApool.tile([128, SA], F32, tag="pA0")
            fA1 = pBpool.tile([128, SA], F32, tag="pA1")
            fB = pCpool.tile([128, BB], F32, tag="pB")
            nc.tensor.matmul(fB[:X_DIM, :], inp[:NP, W2B_O:W2B_O + X_DIM],
                             pt[:, AA:], start=True, stop=False)
            nc.tensor.matmul(fA0[:X_DIM, :], inp[:NP, WFIN_O:WFIN_O + X_DIM],
                             pt[:, :SA], start=True, stop=False)
            nc.tensor.matmul(fA1[:X_DIM, :], inp[:NP, WFIN_O:WFIN_O + X_DIM],
                             pt[:, SA:AA], start=True, stop=False)
            nc.tensor.matmul(fB[:X_DIM, :], inp[:, W1_O:W1_O + X_DIM],
                             zB[:], start=False, stop=True)
            nc.tensor.matmul(fA0[:X_DIM, :], inp[:, W1_O:W1_O + X_DIM],
                             zA0[:], start=False, stop=True)
            nc.tensor.matmul(fA1[:X_DIM, :], inp[:, W1_O:W1_O + X_DIM],
                             zA1[:], start=False, stop=True)
            yo = cpool.tile([X_DIM, BPC], F16, tag="yo")
            nc.scalar.activation(yo[:, :SA], fA0[:X_DIM, :], AF.Copy)
            nc.vector.tensor_copy(yo[:, SA:AA], fA1[:X_DIM, :])
            nc.vector.tensor_scalar(yo[:, AA:], fB[:X_DIM, :], biasF, None,
                                    mybir.AluOpType.add)
            nc.gpsimd.dma_start(yt_d[:], yo[:])

    _legalize_waits(nc)
    return nc


# Barrier/teardown instructions that walrus handles specially; leave alone.
_WAIT_EXEMPT = {"InstEventSemaphore", "InstUnconditionalBranch", "InstCall"}


def _legalize_waits(nc: bass.Bass) -> None:
    """The TPB instruction structs carry a single sync-wait slot, and Tile's
    sem assignment can attach 2+ waits to one instruction (walrus then dies
    with 'Too many sync wait commands').  Fix up the final BIR: drop waits an
    earlier same-engine instruction already guaranteed, and hoist any
    remaining excess waits onto freshly inserted single-wait NoOps."""
    observed: dict[object, dict[int, int]] = {}
    cnt = 0
    for bb in nc.m.functions[0].blocks:
        insts = bb.instructions
        out: list = []
        for ins in insts:
            si = ins.sync_info
            tname = type(ins).__name__
            if si is not None and si.on_wait and tname not in _WAIT_EXEMPT:
                seen = observed.setdefault(ins.engine, {})
                kept = []
                for w in si.on_wait:
                    mono = (w.sync_type == "semaphore"
                            and w.wait_mode == "sem-ge-imm"
                            and w.wait_reg is None)
                    if mono and seen.get(w.id, -1) >= w.wait_value:
                        continue  # engine already waited at least this far
                    kept.append(w)
                    if mono:
                        seen[w.id] = max(seen.get(w.id, -1), w.wait_value)
                while len(kept) > 1:
                    w = kept.pop(0)
                    cnt += 1
                    nop = mybir.InstNoOp(name=f"waitnop-{cnt}", ins=[], outs=[])
                    nop.engine = ins.engine
                    nop.sync_info = mybir.SyncInfo(on_wait=[w], on_update=[])
                    nc.inst_map[nop.name] = nop
                    out.append(nop)
                si.on_wait = kept
            elif si is not None and si.on_wait:
                seen = observed.setdefault(ins.engine, {})
                for w in si.on_wait:
                    if (w.sync_type == "semaphore" and w.wait_mode == "sem-ge-imm"
                            and w.wait_reg is None):
                        seen[w.id] = max(seen.get(w.id, -1), w.wait_value)
            out.append(ins)
        if len(out) != len(insts):
            insts[:] = out


_NC_CACHE: bass.Bass | None = None

# Set by an external harness to enable NTFF tracing; harmless defaults.
TRACE = False
TRACE_DIR: str | None = None
LAST_RESULTS = None


def _get_nc() -> bass.Bass:
    global _NC_CACHE
    if _NC_CACHE is None:
        _NC_CACHE = _build_nc()
    return _NC_CACHE


def kernel(x: np.ndarray, parms: np.ndarray, Q: np.ndarray, A: np.ndarray,
           G: np.ndarray) -> np.ndarray:
    x = np.asarray(x, dtype=np.float32)
    parms = np.asarray(parms, dtype=np.float32)
    wf = _precompute(np.asarray(Q), np.asarray(A), np.asarray(G))

    nc = _get_nc()
    in_maps = []
    for c in range(NCORES):
        lo, hi = c * BPC, (c + 1) * BPC
        pk = wf.copy()
        pk[:N, XT_O:XT_O + BPC] = x[lo:hi].T.astype(np.float16)
        pk[:NP, PT_O:PT_O + BPC] = parms[lo:hi].T.astype(np.float16)
        in_maps.append({"inp": pk})
    global LAST_RESULTS
    kw = {}
    if TRACE:
        kw = {"trace": True, "tmpdir": TRACE_DIR}
    r = run_bass_kernel_spmd(nc, in_maps, list(range(NCORES)), **kw)
    LAST_RESULTS = r
    res = r.results
    out = np.empty((BATCH, X_DIM), dtype=np.float32)
    for c in range(NCORES):
        out[c * BPC:(c + 1) * BPC] = res[c]["yt"].T.astype(np.float32)
    return out


# revision 8
# speedup vs baseline: 1.0120x; 1.0120x over previous
"""Trainium2 Bass kernel for nn_DRSolver (Douglas-Rachford QP solver).

Mathematical collapse: JF and the Hessian are batch-constant, so prox_g1 is
the affine map y = P x + C p and the DR step, in the relu-lifted state
z = [y_top(64); u(32); relu(v)(32)], is

    z' = max(Wx z + Wp p, s),   s = -inf on rows 0:96, 0 on rows 96:128

Device mapping (per core: 512 batch columns, state [128, cols] f16).
Columns split in two fixed regions so both psum-capable engines evacuate in
parallel, ONE instruction each per step:

* Region A (cols 0:AA, VectorE): SHIFTED state zh = z - m with
  m = (I-Wx)^-1 Wp p.  zh' = max(Wx zh, T), T = s - m precomputed — no
  per-step bias matmul; evacuation is one tensor_tensor(max).

* Region B (cols AA:512, ScalarE): OFFSET state zt = z + delta,
  delta = B0 on rows 0:96.  Evacuation is one activation(Relu, bias):
  bias = delta - B0*rowsum(Wx[:, :96]) makes Relu a copy on the
  (always-positive) pass rows and exact max(.,0) on the relu rows, while
  cancelling the Wx*delta term — so region B needs only Wp and Wx matmuls.

Per step: 3 matmuls + 1 DVE op + 1 ACT op.  All inputs ride ONE
gpsimd(SWDGE) DMA — the hardware-DGE queues move ~7-14 GB/s here while the
SW path aggregates packets across all 16 DMA engines (~400 GB/s).  fp32
biases are smuggled in the f16 tensor and bitcast on device.  Dummy matmuls
warm the PE clock gate and a dummy Relu preloads the ACT table during the
DMA wait.
"""

import numpy as np

import concourse.bass as bass
import concourse.tile as tile
import concourse.mybir as mybir
from concourse.bass_utils import run_bass_kernel_spmd

X_DIM, N_INEQ, N_EQ = 64, 32, 16
N = X_DIM + N_INEQ          # 96
M = N_EQ + N_INEQ           # 48
NP = X_DIM + N_EQ + N_INEQ  # 112 (parms dim)
NUM_STEPS = 10
BATCH = 4096
NCORES = 8
BPC = BATCH // NCORES       # 512 samples per core
SA = 184                    # per-chain cols for A0/A1 (VectorE, shifted state)
AA = 2 * SA                 # region A total
BB = BPC - AA               # region B (ScalarE, offset state)
B0 = 32.0                   # pass-row offset for region B (|state| < 26)
NEG_BIG = -60000.0

F32 = mybir.dt.float32
F16 = mybir.dt.float16
AF = mybir.ActivationFunctionType

# packed input tensor column layout (f16); chunk1 = everything step 1 and
# the T build need, chunk2 (W1, WfinA, biases) only matters from step 2 on.
W2B_O, W2A1_O, WFT_O, WMT_O = 0, 128, 256, 384
XT_O = 512                  # x.T           rows 0:96,  512 cols
PT_O = XT_O + BPC           # parms.T       rows 0:112, 512 cols
CHUNK1 = PT_O + BPC
W1_O, WFIN_O = CHUNK1, CHUNK1 + 128
BI_O = WFIN_O + 64          # f32 biases as f16 bit pairs: [bias2 | biasF]
IN_COLS = BI_O + 4


def _precompute(Q: np.ndarray, A: np.ndarray, G: np.ndarray) -> np.ndarray:
    """Host-side factorization collapse (float64); returns the weight+bias
    part of the packed input tensor, [128, IN_COLS] f16 with x/p zeroed."""
    Qd, Ad, Gd = (m.astype(np.float64) for m in (Q, A, G))
    JF = np.zeros((M, N))
    JF[:N_EQ, :X_DIM] = Ad
    JF[N_EQ:, :X_DIM] = Gd
    JF[N_EQ:, X_DIM:] = np.eye(N_INEQ)
    Md = np.eye(N)
    Md[:X_DIM, :X_DIM] += Qd                      # gamma/2 * I + blockdiag(Q,0)
    Qc, _ = np.linalg.qr(JF.T, mode="complete")
    Qn = Qc[:, M:]                                # null-space basis of JF
    S = Qn.T @ Md @ Qn
    P = Qn @ np.linalg.solve(S, Qn.T)
    Z = JF.T @ np.linalg.solve(JF @ JF.T, np.eye(M))  # pinv(JF)
    C = np.zeros((N, NP))
    C[:, :X_DIM] = -P[:, :X_DIM]
    C[:, X_DIM:] = Z - P @ (Md @ Z)

    Es = np.eye(N)[X_DIM:]
    Ps, Cs = P[X_DIM:], C[X_DIM:]
    Wfull = np.concatenate([P[:X_DIM], Es - Ps, 2 * Ps - Es], 0)   # [128, 96]
    Wp_c = np.concatenate([C[:X_DIM], -Cs, 2 * Cs], 0)             # [128, 112]
    L = np.zeros((96, 128))
    L[:96, :96] = np.eye(96)
    L[64:96, 96:128] = np.eye(32)
    Wx = Wfull @ L                                                 # [128, 128]
    Mp = np.linalg.solve(np.eye(128) - Wx, Wp_c)                   # [128, 112]
    wxs = Wx[:, :96].sum(1)

    w = np.zeros((128, IN_COLS), dtype=np.float64)
    w[:128, W1_O:W1_O + 128] = Wx.T
    w[:NP, W2B_O:W2B_O + 128] = Wp_c.T
    w[:NP, W2A1_O:W2A1_O + 128] = (Wp_c - Mp).T
    w[:N, WFT_O:WFT_O + 128] = Wfull.T
    w[:NP, WMT_O + 96:WMT_O + 128] = -Mp[96:128].T   # cols 0:96 stay zero
    w[:NP, WFIN_O:WFIN_O + 64] = Mp[:64].T
    wf = w.astype(np.float16)
    # fp32 biases bit-packed into f16 lanes: [bias2 | biasF]
    delta = np.concatenate([B0 * np.ones(96), np.zeros(32)])
    bias2 = (delta - B0 * wxs).astype(np.float32)       # steps 2..9 (ACT)
    biasF = (-B0 * wxs).astype(np.float32)              # final region B (DVE)
    bb = np.stack([bias2, biasF], 1)                    # [128, 2] f32
    wf[:, BI_O:BI_O + 4] = bb.view(np.float16)
    return wf


def _build_nc() -> bass.Bass:
    nc = bass.Bass()
    in_d = nc.dram_tensor("inp", [128, IN_COLS], F16, kind="ExternalInput")
    yt_d = nc.dram_tensor("yt", [X_DIM, BPC], F16, kind="ExternalOutput")

    with tile.TileContext(nc) as tc:
        with (
            tc.tile_pool(name="sbuf", bufs=1) as cpool,
            tc.tile_pool(name="stA", bufs=2) as sApool,
            tc.tile_pool(name="stB", bufs=2) as sBpool,
            tc.tile_pool(name="psA", bufs=2, space="PSUM") as pApool,
            tc.tile_pool(name="psB", bufs=2, space="PSUM") as pBpool,
            tc.tile_pool(name="psC", bufs=2, space="PSUM") as pCpool,
            tc.tile_pool(name="psm", bufs=1, space="PSUM") as mpool,
        ):
            inp = cpool.tile([128, IN_COLS], F16, tag="inp")
            # SWDGE DMAs (packets aggregate across all 16 DMA engines);
            # chunk1 unblocks the T build and step 1, chunk2 follows.
            nc.gpsimd.dma_start(inp[:, :CHUNK1], in_d[:, :CHUNK1])
            nc.gpsimd.dma_start(inp[:, CHUNK1:], in_d[:, CHUNK1:])
            xt = inp[:N, XT_O:XT_O + BPC]
            pt = inp[:NP, PT_O:PT_O + BPC]
            bias2 = inp[:, BI_O:BI_O + 4].bitcast(F32)[:, 0:1]
            biasF = inp[:X_DIM, BI_O:BI_O + 4].bitcast(F32)[:, 1:2]

            # Constants (engines are free while the DMA is in flight)
            bias1 = cpool.tile([128, 1], F32, tag="bias1")
            nc.vector.memset(bias1[:N, :], B0)
            nc.vector.memset(bias1[N:, :], 0.0)
            Tt = cpool.tile([128, AA], F32, tag="Tt")
            nc.vector.memset(Tt[:N, :], NEG_BIG)

            # Dummy activation: walrus inserts the ACT table load (~2.7us)
            # before this, overlapping the input-DMA wait.
            dum = cpool.tile([1, 1], F32, tag="dum")
            nc.scalar.activation(dum[:], bias1[0:1, :], AF.Relu)

            # HAM warm-up matmuls on a zero scratch tile.
            scr = cpool.tile([128, BPC], F16, tag="scr")
            nc.vector.memset(scr[:], 0.0)
            wps = mpool.tile([128, BPC], F32, tag="warm")
            for _ in range(3):
                nc.tensor.matmul(wps[:], scr[:, :128], scr[:], start=True, stop=True)

            # Region A threshold: T rows 96:128 = -m_v (WMT has zero cols 0:96)
            nc.tensor.matmul(wps[:, :AA], inp[:NP, WMT_O:WMT_O + 128],
                             pt[:, :AA], start=True, stop=True)
            nc.scalar.activation(Tt[N:, :SA], wps[N:, :SA], AF.Copy)
            nc.vector.tensor_copy(Tt[N:, SA:], wps[N:, SA:AA])

            zA0 = zA1 = zB = None
            for k in range(NUM_STEPS - 1):
                pA0 = pApool.tile([128, SA], F32, tag="pA0")
                pA1 = pBpool.tile([128, SA], F32, tag="pA1")
                pB = pCpool.tile([128, BB], F32, tag="pB")
                if k == 0:
                    # z1 = max(Wfull x + c, s); region A gets c - m instead.
                    nc.tensor.matmul(pB[:], inp[:NP, W2B_O:W2B_O + 128],
                                     pt[:, AA:], start=True, stop=False)
                    nc.tensor.matmul(pA0[:], inp[:NP, W2A1_O:W2A1_O + 128],
                                     pt[:, :SA], start=True, stop=False)
                    nc.tensor.matmul(pA1[:], inp[:NP, W2A1_O:W2A1_O + 128],
                                     pt[:, SA:AA], start=True, stop=False)
                    nc.tensor.matmul(pB[:], inp[:N, WFT_O:WFT_O + 128],
                                     xt[:, AA:], start=False, stop=True)
                    nc.tensor.matmul(pA0[:], inp[:N, WFT_O:WFT_O + 128],
                                     xt[:, :SA], start=False, stop=True)
                    nc.tensor.matmul(pA1[:], inp[:N, WFT_O:WFT_O + 128],
                                     xt[:, SA:AA], start=False, stop=True)
                    bact = bias1
                else:
                    nc.tensor.matmul(pB[:], inp[:NP, W2B_O:W2B_O + 128],
                                     pt[:, AA:], start=True, stop=False)
                    nc.tensor.matmul(pA0[:], inp[:, W1_O:W1_O + 128],
                                     zA0[:], start=True, stop=True)
                    nc.tensor.matmul(pA1[:], inp[:, W1_O:W1_O + 128],
                                     zA1[:], start=True, stop=True)
                    nc.tensor.matmul(pB[:], inp[:, W1_O:W1_O + 128],
                                     zB[:], start=False, stop=True)
                    bact = bias2
                zA0n = sApool.tile([128, SA], F16, tag="zA0")
                zA1n = sApool.tile([128, SA], F16, tag="zA1")
                zBn = sBpool.tile([128, BB], F16, tag="zB")
                nc.vector.tensor_tensor(zA0n[:], pA0[:], Tt[:, :SA],
                                        mybir.AluOpType.max)
                nc.vector.tensor_tensor(zA1n[:], pA1[:], Tt[:, SA:],
                                        mybir.AluOpType.max)
                nc.scalar.activation(zBn[:], pB[:], AF.Relu, bias=bact)
                zA0, zA1, zB = zA0n, zA1n, zBn

            # Final step: only y[:64]; region A needs +m_top (= Mp[:64] p),
            # region B needs -B0*wxs[:64] (DVE tensor_scalar, per-partition).
            fA0 = pApool.tile([128, SA], F32, tag="pA0")
            fA1 = pBpool.tile([128, SA], F32, tag="pA1")
            fB = pCpool.tile([128, BB], F32, tag="pB")
            nc.tensor.matmul(fA1[:X_DIM, :], inp[:NP, WFIN_O:WFIN_O + X_DIM],
                             pt[:, SA:AA], start=True, stop=False)
            nc.tensor.matmul(fB[:X_DIM, :], inp[:NP, W2B_O:W2B_O + X_DIM],
                             pt[:, AA:], start=True, stop=False)
            nc.tensor.matmul(fA0[:X_DIM, :], inp[:NP, WFIN_O:WFIN_O + X_DIM],
                             pt[:, :SA], start=True, stop=False)
            nc.tensor.matmul(fA1[:X_DIM, :], inp[:, W1_O:W1_O + X_DIM],
                             zA1[:], start=False, stop=True)
            nc.tensor.matmul(fB[:X_DIM, :], inp[:, W1_O:W1_O + X_DIM],
                             zB[:], start=False, stop=True)
            nc.tensor.matmul(fA0[:X_DIM, :], inp[:, W1_O:W1_O + X_DIM],
                             zA0[:], start=False, stop=True)
            yo = cpool.tile([X_DIM, BPC], F16, tag="yo")
            nc.vector.tensor_copy(yo[:, SA:AA], fA1[:X_DIM, :])
            nc.vector.tensor_scalar(yo[:, AA:], fB[:X_DIM, :], biasF, None,
                                    mybir.AluOpType.add)
            nc.scalar.activation(yo[:, :SA], fA0[:X_DIM, :], AF.Copy)
            nc.gpsimd.dma_start(yt_d[:], yo[:])

    _legalize_waits(nc)
    return nc


# Barrier/teardown instructions that walrus handles specially; leave alone.
_WAIT_EXEMPT = {"InstEventSemaphore", "InstUnconditionalBranch", "InstCall"}


def _legalize_waits(nc: bass.Bass) -> None:
    """The TPB instruction structs carry a single sync-wait slot, and Tile's
    sem assignment can attach 2+ waits to one instruction (walrus then dies
    with 'Too many sync wait commands').  Fix up the final BIR: drop waits an
    earlier same-engine instruction already guaranteed, and hoist any
    remaining excess waits onto freshly inserted single-wait NoOps."""
    observed: dict[object, dict[int, int]] = {}
    cnt = 0
    for bb in nc.m.functions[0].blocks:
        insts = bb.instructions
        out: list = []
        for ins in insts:
            si = ins.sync_info
            tname = type(ins).__name__
            if si is not None and si.on_wait and tname not in _WAIT_EXEMPT:
                seen = observed.setdefault(ins.engine, {})
                kept = []
                for w in si.on_wait:
                    mono = (w.sync_type == "semaphore"
                            and w.wait_mode == "sem-ge-imm"
                            and w.wait_reg is None)
                    if mono and seen.get(w.id, -1) >= w.wait_value:
                        continue  # engine already waited at least this far
                    kept.append(w)
                    if mono:
                        seen[w.id] = max(seen.get(w.id, -1), w.wait_value)
                while len(kept) > 1:
                    w = kept.pop(0)
                    cnt += 1
                    nop = mybir.InstNoOp(name=f"waitnop-{cnt}", ins=[], outs=[])
                    nop.engine = ins.engine
                    nop.sync_info = mybir.SyncInfo(on_wait=[w], on_update=[])
                    nc.inst_map[nop.name] = nop
                    out.append(nop)
                si.on_wait = kept
            elif si is not None and si.on_wait:
                seen = observed.setdefault(ins.engine, {})
                for w in si.on_wait:
                    if (w.sync_type == "semaphore" and w.wait_mode == "sem-ge-imm"
                            and w.wait_reg is None):
                        seen[w.id] = max(seen.get(w.id, -1), w.wait_value)
            out.append(ins)
        if len(out) != len(insts):
            insts[:] = out


_NC_CACHE: bass.Bass | None = None

# Set by an external harness to enable NTFF tracing; harmless defaults.
TRACE = False
TRACE_DIR: str | None = None
LAST_RESULTS = None


def _get_nc() -> bass.Bass:
    global _NC_CACHE
    if _NC_CACHE is None:
        _NC_CACHE = _build_nc()
    return _NC_CACHE


def kernel(x: np.ndarray, parms: np.ndarray, Q: np.ndarray, A: np.ndarray,
           G: np.ndarray) -> np.ndarray:
    x = np.asarray(x, dtype=np.float32)
    parms = np.asarray(parms, dtype=np.float32)
    wf = _precompute(np.asarray(Q), np.asarray(A), np.asarray(G))

    nc = _get_nc()
    in_maps = []
    for c in range(NCORES):
        lo, hi = c * BPC, (c + 1) * BPC
        pk = wf.copy()
        pk[:N, XT_O:XT_O + BPC] = x[lo:hi].T.astype(np.float16)
        pk[:NP, PT_O:PT_O + BPC] = parms[lo:hi].T.astype(np.float16)
        in_maps.append({"inp": pk})
    global LAST_RESULTS
    kw = {}
    if TRACE:
        kw = {"trace": True, "tmpdir": TRACE_DIR}
    r = run_bass_kernel_spmd(nc, in_maps, list(range(NCORES)), **kw)
    LAST_RESULTS = r
    res = r.results
    out = np.empty((BATCH, X_DIM), dtype=np.float32)
    for c in range(NCORES):
        out[c * BPC:(c + 1) * BPC] = res[c]["yt"].T.astype(np.float32)
    return out
